# revision 28
# baseline (speedup 1.0000x reference)
import sys

sys.path.insert(0, "/opt/trn_rl_repo")
import os

import numpy as np

N1, N2, D = 8192, 8192, 256
NCORES = 8
QPC = 2048  # queries per core (4 groups x 2 cores each)
HDB = N1 // 2  # db points per core per side (4096)
TPS = HDB // 128  # db tiles per side (32)
NTILES = 2 * TPS  # 64 (side A = s1 half, side B = s2 half)
PW = 2048  # db dma piece width
NEG = -60000.0  # below any real value, finite in fp16

# Drain/merge roles per [128,1024] half-unit u = 2*t + H:
#   F  : DVE fused  acc = max(psum + nrm, acc)       (scalar_tensor_tensor)
#   AV : Act drain (psum + nrm -> tmp fp16), DVE merge acc = max(acc, tmp)
def _role(u):
    return "F" if u % 4 == 2 else "AV"


def _build_nc():
    import concourse.bass as bass
    import concourse.tile as tile
    from concourse import mybir

    f32, f32r = mybir.dt.float32, mybir.dt.float32r
    f16 = mybir.dt.float16
    add, vmax = mybir.AluOpType.add, mybir.AluOpType.max

    nc = bass.Bass()
    dp = {}  # dram pieces
    for nm, shape in [
        # q<ch>x packs [query-half0 | tile0-1 weights] per chunk so each
        # DMA queue carries ONE critical stream; head0b has the mask +
        # norms (needed ~1us after the first matmul, gpsimd's slow DGE
        # start still makes that)
        ("q0x", [128, 1280]),
        ("q1x", [128, 1280]),
        ("head0b", [128, 320]),
        ("q0h1", [128, 1024]),
        ("q1h1", [128, 1024]),
        ("a00t", [128, 768]),
        ("a00b", [128, 1024]),
        ("a10t", [128, 768]),
        ("a10b", [128, 1024]),
        ("a01", [128, PW]),
        ("a11", [128, PW]),
        ("b00", [128, PW]),
        ("b10", [128, PW]),
        ("b01", [128, PW]),
        ("b11", [128, PW]),
    ]:
        dp[nm] = nc.dram_tensor(nm, shape, f32r, kind="ExternalInput")
    o = nc.dram_tensor("o", [128, 2, QPC], f16, kind="ExternalOutput")

    with tile.TileContext(nc) as tc:
        with (
            tc.tile_pool(name="sb", bufs=1) as sb,
            tc.tile_pool(name="tp", bufs=4) as tp,
            tc.tile_pool(name="ps", bufs=4, space="PSUM") as ps,
        ):
            tsb = {}

            def _dma(nm, eng):
                t = sb.tile(list(dp[nm].shape), f32r, name=nm, tag=nm)
                eng.dma_start(out=t, in_=dp[nm][:])
                tsb[nm] = t

            # All queued DMA transfers share HBM bandwidth concurrently
            # (fair-share), so only the critical head is issued up front,
            # spread over the three DMA-capable queues (sync, scalar,
            # gpsimd). Bulk pieces are issued just-in-time from the Act
            # sequencer inside the tile loop (see below) so they never
            # steal bandwidth from the head.
            # Queues drain their own issues sequentially, so ordering within
            # a queue is a natural priority mechanism: the first-matmul
            # critical set (head0 + both query chunks) leads each queue,
            # the tile 2-7 weights follow right behind.
            _dma("q0x", nc.sync)
            _dma("a00t", nc.sync)
            _dma("q1x", nc.scalar)
            _dma("a10t", nc.scalar)
            _dma("head0b", nc.gpsimd)
            thead = tsb["head0b"]
            tnegi = thead[:, 0:128]
            tposi = thead[:, 128:256]
            # just-in-time bulk issues, keyed by drain-unit emission index
            LATE = {1: ("q0h1", "q1h1"), 3: ("a00b", "a10b"),
                    8: ("a01", "a11"), 36: ("b00", "b10"),
                    60: ("b01", "b11")}

            # separate tiles per half so each output DMA depends only on
            # its own merge chain, not the sibling half's last merge
            accs = {
                k: sb.tile([128, 1024], f16, name=f"acc{k}", tag=f"acc{k}")
                for k in ("A0", "A1", "B0", "B1")
            }
            first = {}

            # PE warmup: streaming 512-col matmuls on a memset tile (no DMA
            # dependency) keep the PE array busy enough to open the HAM
            # clock-gate to 2.4GHz before the first real matmul.
            wsrc = sb.tile([128, 512], f32r, tag="wsrc")
            nc.vector.memset(wsrc.bitcast(f32), 0.0)
            wpst = ps.tile([128, 1024], f32, tag="pst")
            for i in range(8):
                nc.tensor.matmul(
                    out=wpst[:, 0:512],
                    lhsT=wsrc[:, 0:128],
                    rhs=wsrc,
                    start=True,
                    stop=True,
                )

            # Unit schedule: the critical DMA head only covers query-half 0
            # plus the first weight slivers, so tiles 0-15 are processed as
            # half-0 units first, then their half-1 (re-loading weights),
            # then tiles 16-63 with both halves per weight load.
            units = [(t, 0) for t in range(16)]
            units += [(t, 1) for t in range(16)]
            units += [(t, H) for t in range(16, NTILES) for H in range(2)]

            def _w(ch, side, qr, off):
                if side == "a" and qr == 0:
                    if off < 256:
                        return tsb[f"q{ch}x"][:, 1024 + off : 1024 + off + 128]
                    if off < 1024:
                        return tsb[f"a{ch}0t"][:, off - 256 : off - 256 + 128]
                    return tsb[f"a{ch}0b"][:, off - 1024 : off - 1024 + 128]
                return tsb[f"{side}{ch}{qr}"][:, off : off + 128]

            ui = 0
            i_ = 0
            while i_ < len(units):
                t, H0 = units[i_]
                paired = i_ >= 32
                Hs = (0, 1) if paired else (H0,)
                i_ += 2 if paired else 1

                side = "a" if t < TPS else "b"
                S = "A" if t < TPS else "B"
                tl = t if t < TPS else t - TPS
                qr, off = tl // 16, (tl % 16) * 128

                psl = {
                    H: ps.tile([128, 1024], f32, name=f"ps{t}_{H}", tag="pst")
                    for H in Hs
                }
                for ch in range(2):
                    for H in Hs:
                        q = tsb[f"q{ch}x"] if H == 0 else tsb[f"q{ch}h1"]
                        for i in range(2):
                            nc.tensor.matmul(
                                out=psl[H][:, i * 512 : (i + 1) * 512],
                                lhsT=_w(ch, side, qr, off),
                                rhs=q[:, i * 512 : (i + 1) * 512],
                                start=(ch == 0),
                                stop=(ch == 1),
                            )
                    if ch == 0 and t < 16 and (t * 128) // 1024 in Hs:
                        # self-pair mask (negi is all-zero on odd cores)
                        col = t * 128
                        nc.tensor.matmul(
                            out=psl[col // 1024][:, col % 1024 : col % 1024 + 128],
                            lhsT=tnegi[:, 0:128],
                            rhs=tposi[:, 0:128],
                            start=False,
                            stop=False,
                            skip_group_check=True,
                        )

                bias = thead[:, 256 + t : 256 + t + 1].bitcast(f32)
                for H in Hs:
                    if ui in LATE:
                        for nm in LATE[ui]:
                            _dma(nm, nc.scalar)
                    acc = accs[S + str(H)]
                    role = "F" if ui >= 126 else _role(ui)
                    ui += 1
                    if (S, H) not in first:
                        first[(S, H)] = True
                        if role == "F":
                            nc.vector.tensor_scalar_add(
                                out=acc, in0=psl[H], scalar1=bias
                            )
                        else:
                            nc.scalar.add(out=acc, in_=psl[H], add=bias)
                    elif role == "F":
                        nc.vector.scalar_tensor_tensor(
                            out=acc,
                            in0=psl[H],
                            scalar=bias,
                            in1=acc,
                            op0=add,
                            op1=vmax,
                        )
                    else:
                        tmp = tp.tile([128, 1024], f16, tag=f"tmp{H}")
                        nc.scalar.add(out=tmp, in_=psl[H], add=bias)
                        nc.vector.tensor_max(out=acc, in0=acc, in1=tmp)

                if t == TPS - 1 and (paired or H0 == 1):
                    nc.sync.dma_start(out=o[:, 0, 0:1024], in_=accs["A0"])
                    nc.gpsimd.dma_start(out=o[:, 0, 1024:2048], in_=accs["A1"])
            nc.sync.dma_start(out=o[:, 1, 0:1024], in_=accs["B0"])
            nc.gpsimd.dma_start(out=o[:, 1, 1024:2048], in_=accs["B1"])

    from concourse.bass import _bass_rust

    _bass_rust.move_matmul_waits_to_ldweights(nc.m)
    _bass_rust.generate_event_semaphores(nc)
    return nc


def _prep_maps(s1, s2):
    s1T = np.ascontiguousarray(s1.T.reshape(2, 128, N1).transpose(1, 0, 2))
    s1T2 = np.concatenate([s1T, s1T], axis=2)
    s2T = np.ascontiguousarray(s2.T.reshape(2, 128, N2).transpose(1, 0, 2))
    s2T2 = np.concatenate([s2T, s2T], axis=2)

    sq1 = np.square(s1.astype(np.float64)).sum(1)
    sq2 = np.square(s2.astype(np.float64)).sum(1)
    n1h2 = np.concatenate([(-0.5 * sq1).astype(np.float32)] * 2)
    n2h2 = np.concatenate([(-0.5 * sq2).astype(np.float32)] * 2)

    negi = np.zeros((128, 128), dtype=np.float32)
    np.fill_diagonal(negi, NEG)
    zeros = np.zeros((128, 128), dtype=np.float32)
    posi = np.eye(128, dtype=np.float32)

    in_maps = []
    for c in range(NCORES):
        g, h = c // 2, c % 2
        qb = 2048 * g  # query base row
        db = (2048 * g + 4096 * h) % N1  # db base row
        im = {}
        for ch in range(2):
            im[f"q{ch}x"] = np.ascontiguousarray(
                np.concatenate(
                    [s1T2[:, ch, qb : qb + 1024], s1T2[:, ch, db : db + 256]], axis=1
                )
            )
            im[f"q{ch}h1"] = np.ascontiguousarray(s1T2[:, ch, qb + 1024 : qb + 2048])
            im[f"a{ch}0t"] = np.ascontiguousarray(s1T2[:, ch, db + 256 : db + 1024])
            im[f"a{ch}0b"] = np.ascontiguousarray(s1T2[:, ch, db + 1024 : db + 2048])
            im[f"a{ch}1"] = np.ascontiguousarray(s1T2[:, ch, db + 2048 : db + 4096])
            im[f"b{ch}0"] = np.ascontiguousarray(s2T2[:, ch, db : db + 2048])
            im[f"b{ch}1"] = np.ascontiguousarray(s2T2[:, ch, db + 2048 : db + 4096])
        nrmA = np.ascontiguousarray(n1h2[db : db + HDB]).reshape(TPS, 128).T
        nrmB = np.ascontiguousarray(n2h2[db : db + HDB]).reshape(TPS, 128).T
        nrm = np.concatenate([nrmA, nrmB], axis=1)  # [128, 64]
        im["head0b"] = np.ascontiguousarray(
            np.concatenate(
                [negi if h == 0 else zeros, posi, nrm], axis=1
            ).astype(np.float32)
        )
        in_maps.append(im)
    return in_maps, sq1


def kernel(s1, s2, k):
    assert int(k) == 1
    from concourse.bass_utils import run_bass_kernel_spmd

    s1 = np.asarray(s1, dtype=np.float32)
    s2 = np.asarray(s2, dtype=np.float32)
    in_maps, sq1 = _prep_maps(s1, s2)

    nc = _build_nc()
    res = run_bass_kernel_spmd(
        nc,
        in_maps,
        core_ids=list(range(NCORES)),
        trace=os.environ.get("KBENCH_TRACE") == "1",
    )
    kernel.last_results = res

    total = 0.0
    for g in range(4):
        o0 = res.results[2 * g]["o"].astype(np.float64)  # [128, 2, QPC]
        o1 = res.results[2 * g + 1]["o"].astype(np.float64)
        maxA = np.maximum(o0[:, 0, :].max(axis=0), o1[:, 0, :].max(axis=0))
        maxB = np.maximum(o0[:, 1, :].max(axis=0), o1[:, 1, :].max(axis=0))
        sqx = sq1[2048 * g : 2048 * (g + 1)]
        rho_sq = np.maximum(sqx - 2.0 * maxA, 1e-20)
        nu_sq = np.maximum(sqx - 2.0 * maxB, 1e-20)
        total += 0.5 * (np.log(nu_sq) - np.log(rho_sq)).sum()
    base = np.log(N2 / (N1 - 1))
    return np.float32(base + (D / N1) * total)


# revision 29
# speedup vs baseline: 1.0082x; 1.0082x over previous
import sys

sys.path.insert(0, "/opt/trn_rl_repo")
import os

import numpy as np

N1, N2, D = 8192, 8192, 256
NCORES = 8
QPC = 2048  # queries per core (4 groups x 2 cores each)
HDB = N1 // 2  # db points per core per side (4096)
TPS = HDB // 128  # db tiles per side (32)
NTILES = 2 * TPS  # 64 (side A = s1 half, side B = s2 half)
PW = 2048  # db dma piece width
NEG = -60000.0  # below any real value, finite in fp16

# Drain/merge roles per [128,1024] half-unit u = 2*t + H:
#   F  : DVE fused  acc = max(psum + nrm, acc)       (scalar_tensor_tensor)
#   AV : Act drain (psum + nrm -> tmp fp16), DVE merge acc = max(acc, tmp)
def _role(u):
    return "F" if u % 4 == 2 else "AV"


def _build_nc():
    import concourse.bass as bass
    import concourse.tile as tile
    from concourse import mybir

    f32, f32r = mybir.dt.float32, mybir.dt.float32r
    f16 = mybir.dt.float16
    add, vmax = mybir.AluOpType.add, mybir.AluOpType.max

    nc = bass.Bass()
    dp = {}  # dram pieces
    for nm, shape in [
        # q<ch>x packs [query-half0 | tile0-1 weights] per chunk so each
        # DMA queue carries ONE critical stream; head0b has the mask +
        # norms (needed ~1us after the first matmul, gpsimd's slow DGE
        # start still makes that)
        ("q0x", [128, 1280]),
        ("q1x", [128, 1280]),
        ("head0b", [128, 320]),
        ("q0h1", [128, 1024]),
        ("q1h1", [128, 1024]),
        ("a00t", [128, 768]),
        ("a00b", [128, 1024]),
        ("a10t", [128, 768]),
        ("a10b", [128, 1024]),
        ("a01", [128, PW]),
        ("a11", [128, PW]),
        ("b00", [128, PW]),
        ("b10", [128, PW]),
        ("b01", [128, PW]),
        ("b11", [128, PW]),
    ]:
        dp[nm] = nc.dram_tensor(nm, shape, f32r, kind="ExternalInput")
    o = nc.dram_tensor("o", [128, 2, QPC], f16, kind="ExternalOutput")

    with tile.TileContext(nc) as tc:
        with (
            tc.tile_pool(name="sb", bufs=1) as sb,
            tc.tile_pool(name="tp", bufs=4) as tp,
            tc.tile_pool(name="ps", bufs=4, space="PSUM") as ps,
        ):
            tsb = {}

            def _dma(nm, eng):
                t = sb.tile(list(dp[nm].shape), f32r, name=nm, tag=nm)
                eng.dma_start(out=t, in_=dp[nm][:])
                tsb[nm] = t

            # All queued DMA transfers share HBM bandwidth concurrently
            # (fair-share), so only the critical head is issued up front,
            # spread over the three DMA-capable queues (sync, scalar,
            # gpsimd). Bulk pieces are issued just-in-time from the Act
            # sequencer inside the tile loop (see below) so they never
            # steal bandwidth from the head.
            # Queues drain their own issues sequentially, so ordering within
            # a queue is a natural priority mechanism: the first-matmul
            # critical set (head0 + both query chunks) leads each queue,
            # the tile 2-7 weights follow right behind.
            _dma("q0x", nc.sync)
            _dma("a00t", nc.sync)
            _dma("q1x", nc.scalar)
            _dma("a10t", nc.scalar)
            _dma("head0b", nc.gpsimd)
            thead = tsb["head0b"]
            tnegi = thead[:, 0:128]
            tposi = thead[:, 128:256]
            # just-in-time bulk issues, keyed by drain-unit emission index
            LATE = {1: ("q0h1", "q1h1"), 3: ("a00b", "a10b"),
                    8: ("a01", "a11"), 36: ("b00", "b10"),
                    60: ("b01", "b11")}

            accs = {
                k: sb.tile([128, QPC], f16, name=f"acc{k}", tag=f"acc{k}")
                for k in ("A", "B")
            }
            first = {}

            # PE warmup: streaming 512-col matmuls on a memset tile (no DMA
            # dependency) keep the PE array busy enough to open the HAM
            # clock-gate to 2.4GHz before the first real matmul.
            wsrc = sb.tile([128, 512], f32r, tag="wsrc")
            nc.vector.memset(wsrc.bitcast(f32), 0.0)
            wpst = ps.tile([128, 1024], f32, tag="pst")
            for i in range(8):
                nc.tensor.matmul(
                    out=wpst[:, 0:512],
                    lhsT=wsrc[:, 0:128],
                    rhs=wsrc,
                    start=True,
                    stop=True,
                )

            # Unit schedule: the critical DMA head only covers query-half 0
            # plus the first weight slivers, so tiles 0-15 are processed as
            # half-0 units first, then their half-1 (re-loading weights),
            # then tiles 16-63 with both halves per weight load.
            units = [(t, 0) for t in range(16)]
            units += [(t, 1) for t in range(16)]
            units += [(t, H) for t in range(16, NTILES) for H in range(2)]

            def _w(ch, side, qr, off):
                if side == "a" and qr == 0:
                    if off < 256:
                        return tsb[f"q{ch}x"][:, 1024 + off : 1024 + off + 128]
                    if off < 1024:
                        return tsb[f"a{ch}0t"][:, off - 256 : off - 256 + 128]
                    return tsb[f"a{ch}0b"][:, off - 1024 : off - 1024 + 128]
                return tsb[f"{side}{ch}{qr}"][:, off : off + 128]

            ui = 0
            i_ = 0
            while i_ < len(units):
                t, H0 = units[i_]
                paired = i_ >= 32
                Hs = (0, 1) if paired else (H0,)
                i_ += 2 if paired else 1

                side = "a" if t < TPS else "b"
                S = "A" if t < TPS else "B"
                tl = t if t < TPS else t - TPS
                qr, off = tl // 16, (tl % 16) * 128

                psl = {
                    H: ps.tile([128, 1024], f32, name=f"ps{t}_{H}", tag="pst")
                    for H in Hs
                }
                for ch in range(2):
                    for H in Hs:
                        q = tsb[f"q{ch}x"] if H == 0 else tsb[f"q{ch}h1"]
                        for i in range(2):
                            nc.tensor.matmul(
                                out=psl[H][:, i * 512 : (i + 1) * 512],
                                lhsT=_w(ch, side, qr, off),
                                rhs=q[:, i * 512 : (i + 1) * 512],
                                start=(ch == 0),
                                stop=(ch == 1),
                            )
                    if ch == 0 and t < 16 and (t * 128) // 1024 in Hs:
                        # self-pair mask (negi is all-zero on odd cores)
                        col = t * 128
                        nc.tensor.matmul(
                            out=psl[col // 1024][:, col % 1024 : col % 1024 + 128],
                            lhsT=tnegi[:, 0:128],
                            rhs=tposi[:, 0:128],
                            start=False,
                            stop=False,
                            skip_group_check=True,
                        )

                bias = thead[:, 256 + t : 256 + t + 1].bitcast(f32)
                for H in Hs:
                    if ui in LATE:
                        for nm in LATE[ui]:
                            _dma(nm, nc.scalar)
                    acc = accs[S][:, H * 1024 : (H + 1) * 1024]
                    role = "F" if ui >= 126 else _role(ui)
                    ui += 1
                    if (S, H) not in first:
                        first[(S, H)] = True
                        if role == "F":
                            nc.vector.tensor_scalar_add(
                                out=acc, in0=psl[H], scalar1=bias
                            )
                        else:
                            nc.scalar.add(out=acc, in_=psl[H], add=bias)
                    elif role == "F":
                        nc.vector.scalar_tensor_tensor(
                            out=acc,
                            in0=psl[H],
                            scalar=bias,
                            in1=acc,
                            op0=add,
                            op1=vmax,
                        )
                    else:
                        tmp = tp.tile([128, 1024], f16, tag=f"tmp{H}")
                        nc.scalar.add(out=tmp, in_=psl[H], add=bias)
                        nc.vector.tensor_max(out=acc, in0=acc, in1=tmp)

                if t == TPS - 1 and (paired or H0 == 1):
                    nc.sync.dma_start(out=o[:, 0, 0:1024], in_=accs["A"][:, 0:1024])
                    nc.gpsimd.dma_start(
                        out=o[:, 0, 1024:2048], in_=accs["A"][:, 1024:2048]
                    )
            nc.sync.dma_start(out=o[:, 1, 0:1024], in_=accs["B"][:, 0:1024])
            nc.gpsimd.dma_start(out=o[:, 1, 1024:2048], in_=accs["B"][:, 1024:2048])

    from concourse.bass import _bass_rust

    _bass_rust.move_matmul_waits_to_ldweights(nc.m)
    _bass_rust.generate_event_semaphores(nc)
    return nc


def _prep_maps(s1, s2):
    s1T = np.ascontiguousarray(s1.T.reshape(2, 128, N1).transpose(1, 0, 2))
    s1T2 = np.concatenate([s1T, s1T], axis=2)
    s2T = np.ascontiguousarray(s2.T.reshape(2, 128, N2).transpose(1, 0, 2))
    s2T2 = np.concatenate([s2T, s2T], axis=2)

    sq1 = np.square(s1.astype(np.float64)).sum(1)
    sq2 = np.square(s2.astype(np.float64)).sum(1)
    n1h2 = np.concatenate([(-0.5 * sq1).astype(np.float32)] * 2)
    n2h2 = np.concatenate([(-0.5 * sq2).astype(np.float32)] * 2)

    negi = np.zeros((128, 128), dtype=np.float32)
    np.fill_diagonal(negi, NEG)
    zeros = np.zeros((128, 128), dtype=np.float32)
    posi = np.eye(128, dtype=np.float32)

    in_maps = []
    for c in range(NCORES):
        g, h = c // 2, c % 2
        qb = 2048 * g  # query base row
        db = (2048 * g + 4096 * h) % N1  # db base row
        im = {}
        for ch in range(2):
            im[f"q{ch}x"] = np.ascontiguousarray(
                np.concatenate(
                    [s1T2[:, ch, qb : qb + 1024], s1T2[:, ch, db : db + 256]], axis=1
                )
            )
            im[f"q{ch}h1"] = np.ascontiguousarray(s1T2[:, ch, qb + 1024 : qb + 2048])
            im[f"a{ch}0t"] = np.ascontiguousarray(s1T2[:, ch, db + 256 : db + 1024])
            im[f"a{ch}0b"] = np.ascontiguousarray(s1T2[:, ch, db + 1024 : db + 2048])
            im[f"a{ch}1"] = np.ascontiguousarray(s1T2[:, ch, db + 2048 : db + 4096])
            im[f"b{ch}0"] = np.ascontiguousarray(s2T2[:, ch, db : db + 2048])
            im[f"b{ch}1"] = np.ascontiguousarray(s2T2[:, ch, db + 2048 : db + 4096])
        nrmA = np.ascontiguousarray(n1h2[db : db + HDB]).reshape(TPS, 128).T
        nrmB = np.ascontiguousarray(n2h2[db : db + HDB]).reshape(TPS, 128).T
        nrm = np.concatenate([nrmA, nrmB], axis=1)  # [128, 64]
        im["head0b"] = np.ascontiguousarray(
            np.concatenate(
                [negi if h == 0 else zeros, posi, nrm], axis=1
            ).astype(np.float32)
        )
        in_maps.append(im)
    return in_maps, sq1


def kernel(s1, s2, k):
    assert int(k) == 1
    from concourse.bass_utils import run_bass_kernel_spmd

    s1 = np.asarray(s1, dtype=np.float32)
    s2 = np.asarray(s2, dtype=np.float32)
    in_maps, sq1 = _prep_maps(s1, s2)

    nc = _build_nc()
    res = run_bass_kernel_spmd(
        nc,
        in_maps,
        core_ids=list(range(NCORES)),
        trace=os.environ.get("KBENCH_TRACE") == "1",
    )
    kernel.last_results = res

    total = 0.0
    for g in range(4):
        o0 = res.results[2 * g]["o"].astype(np.float64)  # [128, 2, QPC]
        o1 = res.results[2 * g + 1]["o"].astype(np.float64)
        maxA = np.maximum(o0[:, 0, :].max(axis=0), o1[:, 0, :].max(axis=0))
        maxB = np.maximum(o0[:, 1, :].max(axis=0), o1[:, 1, :].max(axis=0))
        sqx = sq1[2048 * g : 2048 * (g + 1)]
        rho_sq = np.maximum(sqx - 2.0 * maxA, 1e-20)
        nu_sq = np.maximum(sqx - 2.0 * maxB, 1e-20)
        total += 0.5 * (np.log(nu_sq) - np.log(rho_sq)).sum()
    base = np.log(N2 / (N1 - 1))
    return np.float32(base + (D / N1) * total)
